# revision 24
# baseline (speedup 1.0000x reference)
"""Attention-pooling kernel (AttLayer) for Trainium2, data-parallel over batch
across 8 NeuronCores.

  uit = tanh(x @ W + b)            [B, T, A]
  ait = exp(uit @ u) * mask        [B, T]
  out = einsum('btd,bt->bd', x, ait / (sum_t ait + eps))

Shapes hardcoded: x [64, 4096, 256] f32, W [256, 32], b [32], u [32, 1],
mask [64, 4096] bool. Each core handles 8 batches.

v2 design (PE-lean, DMA-bound target):
- x loads are SWDGE cast-DMAs (f32 DRAM -> bf16 SBUF), one [128,16,256]
  slab per half-batch "supergroup" (SG, 2048 t). t = 2048 g + 16 p + r.
- Per chunk (g, r): two bf16 [128,128] PE transposes -> xt (d on partitions).
- x@W uses W as the *stationary* operand: per quad q (4 chunks), matmuls
  write uit^T [32 a, 512 (c s)] at PSUM partitions 32q via
  tile_position=(0, 32q), accumulating two d-halves. One SG fills a full
  [128, 512] PSUM tile (4 quads stacked), so tanh(+bias b4) runs on all
  128 partitions in ONE activation op.
- Scores: per c, matmul(lhsT=tanh_sb[:, 128c:+128], rhs=u4) contracts over
  a within each 32-row quad block (u4 is block-diagonal u), yielding
  [128 slot, 4 quad] -- t lands on partitions, exactly phase-3's layout.
- mask bias add (DVE), exp with fused row-sum (ACT), denominator via a
  [128,1] x [128,1] matmul, phase 3 and output identical to v1.
"""

import os
import sys

sys.path.insert(0, "/opt/trn_rl_repo")

import numpy as np

import concourse.bass as bass
import concourse.mybir as mybir
import concourse.tile as tile
from concourse import bacc
from concourse.bass import ds, ts
from concourse import bass_utils
from concourse.bass_utils import run_bass_kernel_spmd

F32 = mybir.dt.float32
BF16 = mybir.dt.bfloat16

N_CORES = 8
B, T, D, A = 64, 4096, 256, 32
BPC = B // N_CORES          # batches per core
NCH = T // 128              # 128-row chunks per batch (32)
NG = 2                      # supergroups (DMA slabs) per batch
RPG = NCH // NG             # chunks per supergroup (16)
NQ = 4                      # quads per supergroup
EPS = 1e-7
MASK_BIAS = 30.0            # additive pre-exp mask: s + (mask-1)*30

last_exec_time_ns = None
last_result = None


DEBUG = bool(int(os.environ.get("BASS_V2_DEBUG", "0")))


def _build():
    nc = bacc.Bacc(None, target_bir_lowering=False, debug=True)

    x_dram = nc.dram_tensor("x", [BPC, T, D], F32, kind="ExternalInput")
    w2_dram = nc.dram_tensor("w2", [128, 2 * A], F32, kind="ExternalInput")
    u4_dram = nc.dram_tensor("u4", [128, NQ], F32, kind="ExternalInput")
    sel2_dram = nc.dram_tensor("sel2", [128, 2], F32, kind="ExternalInput")
    b4_dram = nc.dram_tensor("b4", [128, 1], F32, kind="ExternalInput")
    maskb_dram = nc.dram_tensor("maskb", [BPC, 128, NCH], F32, kind="ExternalInput")
    ident_dram = nc.dram_tensor("ident", [128, 128], F32, kind="ExternalInput")
    out_dram = nc.dram_tensor("out", [BPC, D], F32, kind="ExternalOutput")
    if DEBUG:
        dbg_xt = nc.dram_tensor("dbg_xt", [128, 512], F32, kind="ExternalOutput")
        dbg_uit = nc.dram_tensor("dbg_uit", [128, 512], F32, kind="ExternalOutput")
        dbg_tanh = nc.dram_tensor("dbg_tanh", [128, 512], F32, kind="ExternalOutput")
        dbg_sm = nc.dram_tensor("dbg_sm", [2, 128, NCH], F32, kind="ExternalOutput")
        dbg_ebf = nc.dram_tensor("dbg_ebf", [2, 128, NCH], F32, kind="ExternalOutput")
        dbg_uit5 = nc.dram_tensor("dbg_uit5", [128, 512], F32, kind="ExternalOutput")
        dbg_o2 = nc.dram_tensor("dbg_o2", [2, 2, 2 * D], F32, kind="ExternalOutput")
        dbg_den = nc.dram_tensor("dbg_den", [2, 1, 1], F32, kind="ExternalOutput")

    with tile.TileContext(nc) as tc:
        with (
            tc.tile_pool(name="const", bufs=1) as cpool,
            tc.tile_pool(name="xb", bufs=6) as xbpool,
            tc.tile_pool(name="xt", bufs=5) as xtpool,
            tc.tile_pool(name="th", bufs=3) as thpool,
            tc.tile_pool(name="small", bufs=2) as spool,
            tc.tile_pool(name="xtps", bufs=2, space="PSUM") as xtpspool,
            tc.tile_pool(name="uitps", bufs=2, space="PSUM") as uitpool,
            tc.tile_pool(name="eps", bufs=1, space="PSUM") as epool,
            tc.tile_pool(name="ops", bufs=2, space="PSUM") as opool,
            tc.tile_pool(name="denps", bufs=1, space="PSUM") as denpool,
        ):
            # ---- constants (one-time) ----
            w2_f = cpool.tile([128, 2 * A], F32, name="w2_f")
            nc.sync.dma_start(out=w2_f[:], in_=w2_dram[:])
            w2_bf = cpool.tile([128, 2 * A], BF16, name="w2_bf")
            nc.vector.tensor_copy(w2_bf[:], w2_f[:])

            u4_f = cpool.tile([128, NQ], F32, name="u4_f")
            nc.sync.dma_start(out=u4_f[:], in_=u4_dram[:])
            u4_bf = cpool.tile([128, NQ], BF16, name="u4_bf")
            nc.vector.tensor_copy(u4_bf[:], u4_f[:])

            sel2 = cpool.tile([128, 2], F32, name="sel2")
            nc.sync.dma_start(out=sel2[:], in_=sel2_dram[:])

            b4 = cpool.tile([128, 1], F32, name="b4")
            nc.sync.dma_start(out=b4[:], in_=b4_dram[:])

            ident = cpool.tile([128, 128], F32, name="ident")
            nc.sync.dma_start(out=ident[:], in_=ident_dram[:])
            ident_bf = cpool.tile([128, 128], BF16, name="ident_bf")
            nc.vector.tensor_copy(ident_bf[:], ident[:])

            ones_f = cpool.tile([128, 1], F32, name="ones_f")
            nc.vector.memset(ones_f[:], 1.0)

            # deferred-emission state for software pipelining
            pend_score = [None]   # (tanh_sb, bb, g)
            e_tiles = {}          # bb -> e_ps tile, allocated at first write

            def emit_score(work):
                tanh_sb, bb, g = work
                # allocate e_ps(bb) lazily HERE so every reference to the
                # previous batch's e_ps tile is already emitted when the pool
                # rotates (bufs=1) -- else the tail's read races next scores.
                if bb not in e_tiles:
                    e_tiles[bb] = epool.tile([128, NCH], F32, name="e_ps", tag="e")
                e_v = e_tiles[bb].rearrange("p (g q c) -> p g q c", g=NG, q=NQ)
                for c in range(4):
                    nc.tensor.matmul(
                        e_v[:, g, :, c],
                        lhsT=tanh_sb[:, ds(128 * c, 128)],
                        rhs=u4_bf[:],
                        start=True,
                        stop=True,
                    )

            def emit_batch_dma(bb):
                """One cast-DMA per batch: [128, 32 chunks, 256] bf16,
                chunk col = 16g + r, t = 2048g + 16p + r."""
                x_bf = xbpool.tile([128, NCH, D], BF16, name="x_bf", tag="xb")
                x_view = x_dram[bb].rearrange(
                    "(g p r) d -> p (g r) d", g=NG, p=128
                )
                if bb == 0:
                    for cq in range(8):
                        nc.gpsimd.dma_start(
                            out=x_bf[:, ds(4 * cq, 4), :],
                            in_=x_view[:, ds(4 * cq, 4), :],
                        )
                else:
                    nc.gpsimd.dma_start(out=x_bf[:], in_=x_view)
                return x_bf

            def emit_sg(bb, g, x_bf):
                """One supergroup: transposes, x@W, tanh."""
                uit_ps = uitpool.tile([128, 4 * 128], F32, name="uit_ps", tag="uit")
                # pair-trick transposes: one f32 [128,128] PE transpose moves a
                # whole [128t, 256d] bf16 chunk (adjacent bf16 pairs ride as one
                # f32; fp32r mantissa loss is ~2^-16 of the pair max -- far
                # below bf16 noise).  xt[P, 2s+j] = x[t_s, 2P+j].
                x_pair = x_bf.rearrange("p r d -> p (r d)").bitcast(F32)
                xt_sbs = []
                for q in range(NQ):
                    xt_ps = xtpspool.tile([128, 4, 128], F32, name="xt_ps", tag="xtps")
                    for cc in range(4):
                        r = RPG * g + 4 * q + cc
                        nc.tensor.transpose(
                            xt_ps[:, cc, :], x_pair[:, ds(128 * r, 128)], ident[:]
                        )
                    xt_sb = xtpool.tile([128, 4, 128], F32, name="xt_sb", tag="xt")
                    if q % 2 == 0:
                        nc.vector.tensor_copy(xt_sb[:], xt_ps[:])
                    else:
                        nc.scalar.copy(xt_sb[:], xt_ps[:])
                    xt_sbs.append(xt_sb[:].bitcast(BF16))  # [128, 4, 256]

                # deferred from previous SG: its score matmuls (tanh is ready)
                if pend_score[0] is not None:
                    emit_score(pend_score[0])

                # x@W: quad q accumulates uit^T into PSUM partitions 32q via
                # tile_position col-groups; q is the inner loop so the four
                # col-groups stream concurrently.  The start clear resets
                # has_written for the WRITTEN PARTITIONS across the full bank
                # width, so each quad's first matmul must set start=True.
                for j in range(2):
                    for cc in range(4):
                        for q in range(NQ):
                            nc.tensor.matmul(
                                uit_ps[ds(32 * q, 32), ds(128 * cc, 128)],
                                lhsT=w2_bf[:, ds(A * j, A)],
                                rhs=xt_sbs[q][:, cc, ds(j, 128, step=2)],
                                start=(j == 0 and cc == 0),
                                stop=(j == 1 and cc == 3 and q == NQ - 1),
                                tile_position=(0, 32 * q),
                            )

                tanh_sb = thpool.tile([128, 4 * 128], BF16, name="tanh_sb", tag="th")
                nc.scalar.activation(
                    tanh_sb[:],
                    uit_ps[:],
                    mybir.ActivationFunctionType.Tanh,
                    bias=b4[:],
                )
                if DEBUG and bb == 0 and g == 0:
                    t1 = spool.tile([128, 512], F32, name="dbg1", tag="dbg1")
                    nc.vector.tensor_copy(
                        t1[:], xt_sbs[0].bitcast(F32).rearrange("p c s -> p (c s)")
                    )
                    nc.sync.dma_start(out=dbg_xt[:], in_=t1[:])
                    t2 = spool.tile([128, 512], F32, name="dbg2", tag="dbg2")
                    nc.vector.tensor_copy(t2[:], uit_ps[:])
                    nc.sync.dma_start(out=dbg_uit[:], in_=t2[:])
                    t3 = spool.tile([128, 512], F32, name="dbg3", tag="dbg3")
                    nc.vector.tensor_copy(t3[:], tanh_sb[:])
                    nc.sync.dma_start(out=dbg_tanh[:], in_=t3[:])
                if DEBUG and bb == 5 and g == 0:
                    t2b = spool.tile([128, 512], F32, name="dbg2b", tag="dbg2b")
                    nc.vector.tensor_copy(t2b[:], uit_ps[:])
                    nc.sync.dma_start(out=dbg_uit5[:], in_=t2b[:])
                pend_score[0] = (tanh_sb, bb, g)
                return x_bf

            def emit_tail(bb, x_bf_tiles):
                e_ps = e_tiles.pop(bb)
                maskb = spool.tile([128, NCH], F32, name="maskb", tag="maskb")
                nc.sync.dma_start(out=maskb[:], in_=maskb_dram[bb])
                s_m = spool.tile([128, NCH], F32, name="s_m", tag="s_m")
                nc.vector.tensor_add(s_m[:], e_ps[:], maskb[:])

                e_bf = spool.tile([128, NCH], BF16, name="e_bf", tag="e_bf")
                er = spool.tile([128, 1], F32, name="er", tag="er")
                nc.scalar.activation(
                    e_bf[:],
                    s_m[:],
                    mybir.ActivationFunctionType.Exp,
                    accum_out=er[:],
                )
                if DEBUG and bb in (0, 5):
                    ii = 0 if bb == 0 else 1
                    nc.sync.dma_start(out=dbg_sm[ii], in_=s_m[:])
                    t5 = spool.tile([128, NCH], F32, name="dbg5", tag="dbg5")
                    nc.vector.tensor_copy(t5[:], e_bf[:])
                    nc.sync.dma_start(out=dbg_ebf[ii], in_=t5[:])

                den_ps = denpool.tile([1, 1], F32, name="den_ps", tag="den")
                nc.tensor.matmul(
                    den_ps[:], lhsT=er[:], rhs=ones_f[:], start=True, stop=True
                )

                # phase 3: weighted sum, two chunks per matmul
                o_ps = opool.tile([2, 2 * D], F32, name="o_ps", tag="o")
                for Q in range(NCH // 2):
                    g, r0 = divmod(2 * Q, RPG)
                    nc.tensor.matmul(
                        o_ps[:],
                        lhsT=e_bf[:, ds(2 * Q, 2)],
                        rhs=x_bf_tiles[g][:, r0 : r0 + 2, :],
                        start=(Q == 0),
                        stop=(Q == NCH // 2 - 1),
                    )

                den_sb = spool.tile([1, 1], F32, name="den_sb", tag="den_sb")
                nc.vector.tensor_scalar_add(den_sb[:], den_ps[:], EPS)
                inv = spool.tile([1, 1], F32, name="inv", tag="inv")
                nc.vector.reciprocal(inv[:], den_sb[:])
                o2_sb = spool.tile([2, 2 * D], F32, name="o2_sb", tag="o2_sb")
                nc.vector.tensor_copy(o2_sb[:], o_ps[:])
                o_hi = spool.tile([1, D], F32, name="o_hi", tag="o_hi")
                nc.sync.dma_start(out=o_hi[:], in_=o2_sb[1:2, ds(D, D)])
                o_sum = spool.tile([1, D], F32, name="o_sum", tag="o_sum")
                nc.vector.tensor_add(o_sum[:], o2_sb[0:1, 0:D], o_hi[:])
                o_sb = spool.tile([1, D], F32, name="o_sb", tag="o_sb")
                nc.vector.tensor_scalar_mul(o_sb[:], o_sum[:], inv[:])
                nc.sync.dma_start(out=out_dram[bb][None, :], in_=o_sb[:])
                if DEBUG and bb in (0, 5):
                    ii = 0 if bb == 0 else 1
                    nc.sync.dma_start(out=dbg_o2[ii], in_=o2_sb[:])
                    nc.sync.dma_start(out=dbg_den[ii], in_=den_sb[:])

            # pipeline: SG(bb,0) [emits scores of bb-1,g1] -> tail(bb-1)
            # -> SG(bb,1) [allocates e_ps(bb), emits scores of bb,g0]
            prev_xbs = None
            for bb in range(BPC):
                xb0 = emit_sg(bb, 0)
                if prev_xbs is not None:
                    emit_tail(bb - 1, prev_xbs)
                xb1 = emit_sg(bb, 1)
                prev_xbs = [xb0, xb1]
            emit_score(pend_score[0])
            emit_tail(BPC - 1, prev_xbs)

    nc.finalize()
    return nc


def kernel(x, mask, W, b, u):
    global last_exec_time_ns, last_result
    x = np.ascontiguousarray(np.asarray(x), dtype=np.float32)
    mask_f = np.asarray(mask).astype(np.float32)
    W = np.asarray(W, dtype=np.float32)
    b = np.asarray(b, dtype=np.float32)
    u = np.asarray(u, dtype=np.float32)

    # host-side layout prep (all tiny; x is only view-sliced)
    # d-parity packing to match pair-trick transposes:
    # w2[p, A*j + a] = W[2p + j, a]
    w2 = np.ascontiguousarray(W.reshape(128, 2 * A))
    # block-diagonal u: u4[32q + a, q] = u[a]
    u4 = np.zeros((128, NQ), dtype=np.float32)
    for q in range(NQ):
        u4[32 * q : 32 * q + 32, q] = u[:, 0]
    # bias replicated per quad row-block
    b4 = np.ascontiguousarray(np.tile(b, NQ)[:, None])
    # mask -> additive pre-exp bias, laid out [b][p][(g r)] with t = 2048g+16p+r
    maskb = np.ascontiguousarray(
        ((mask_f - 1.0) * MASK_BIAS)
        .reshape(B, NG, 128, RPG)
        .transpose(0, 2, 1, 3)
        .reshape(B, 128, NCH)
    )
    ident = np.eye(128, dtype=np.float32)

    nc = _build()

    in_maps = []
    for c in range(N_CORES):
        in_maps.append(
            {
                "x": x[c * BPC : (c + 1) * BPC],
                "w2": w2,
                "u4": u4,
                "b4": b4,
                "maskb": maskb[c * BPC : (c + 1) * BPC],
                "ident": ident,
            }
        )

    trace = bool(int(os.environ.get("BASS_KERNEL_TRACE", "0")))
    res = run_bass_kernel_spmd(
        nc, in_maps, core_ids=list(range(N_CORES)), trace=trace
    )
    last_exec_time_ns = res.exec_time_ns
    last_result = res

    out = np.empty((B, D), dtype=np.float32)
    for c in range(N_CORES):
        out[c * BPC : (c + 1) * BPC] = res.results[c]["out"]
    return out


# revision 26
# speedup vs baseline: 1.1130x; 1.1130x over previous
"""Attention-pooling kernel (AttLayer) for Trainium2, data-parallel over batch
across 8 NeuronCores.

  uit = tanh(x @ W + b)            [B, T, A]
  ait = exp(uit @ u) * mask        [B, T]
  out = einsum('btd,bt->bd', x, ait / (sum_t ait + eps))

Shapes hardcoded: x [64, 4096, 256] f32, W [256, 32], b [32], u [32, 1],
mask [64, 4096] bool. Each core handles 8 batches.

v2 design (PE-lean, DMA-bound target):
- x loads are SWDGE cast-DMAs (f32 DRAM -> bf16 SBUF), one [128,16,256]
  slab per half-batch "supergroup" (SG, 2048 t). t = 2048 g + 16 p + r.
- Per chunk (g, r): two bf16 [128,128] PE transposes -> xt (d on partitions).
- x@W uses W as the *stationary* operand: per quad q (4 chunks), matmuls
  write uit^T [32 a, 512 (c s)] at PSUM partitions 32q via
  tile_position=(0, 32q), accumulating two d-halves. One SG fills a full
  [128, 512] PSUM tile (4 quads stacked), so tanh(+bias b4) runs on all
  128 partitions in ONE activation op.
- Scores: per c, matmul(lhsT=tanh_sb[:, 128c:+128], rhs=u4) contracts over
  a within each 32-row quad block (u4 is block-diagonal u), yielding
  [128 slot, 4 quad] -- t lands on partitions, exactly phase-3's layout.
- mask bias add (DVE), exp with fused row-sum (ACT), denominator via a
  [128,1] x [128,1] matmul, phase 3 and output identical to v1.
"""

import os
import sys

sys.path.insert(0, "/opt/trn_rl_repo")

import numpy as np

import concourse.bass as bass
import concourse.mybir as mybir
import concourse.tile as tile
from concourse import bacc
from concourse.bass import ds, ts
from concourse import bass_utils
from concourse.bass_utils import run_bass_kernel_spmd

F32 = mybir.dt.float32
BF16 = mybir.dt.bfloat16

N_CORES = 8
B, T, D, A = 64, 4096, 256, 32
BPC = B // N_CORES          # batches per core
NCH = T // 128              # 128-row chunks per batch (32)
NG = 2                      # supergroups (DMA slabs) per batch
RPG = NCH // NG             # chunks per supergroup (16)
NQ = 4                      # quads per supergroup
EPS = 1e-7
MASK_BIAS = 30.0            # additive pre-exp mask: s + (mask-1)*30

last_exec_time_ns = None
last_result = None


DEBUG = bool(int(os.environ.get("BASS_V2_DEBUG", "0")))


def _build():
    nc = bacc.Bacc(None, target_bir_lowering=False, debug=True)

    x_dram = nc.dram_tensor("x", [BPC, T, D], F32, kind="ExternalInput")
    w2_dram = nc.dram_tensor("w2", [128, 2 * A], F32, kind="ExternalInput")
    u4_dram = nc.dram_tensor("u4", [128, NQ], F32, kind="ExternalInput")
    sel2_dram = nc.dram_tensor("sel2", [128, 2], F32, kind="ExternalInput")
    b4_dram = nc.dram_tensor("b4", [128, 1], F32, kind="ExternalInput")
    maskb_dram = nc.dram_tensor("maskb", [BPC, 128, NCH], F32, kind="ExternalInput")
    ident_dram = nc.dram_tensor("ident", [128, 128], F32, kind="ExternalInput")
    out_dram = nc.dram_tensor("out", [BPC, D], F32, kind="ExternalOutput")
    if DEBUG:
        dbg_xt = nc.dram_tensor("dbg_xt", [128, 512], F32, kind="ExternalOutput")
        dbg_uit = nc.dram_tensor("dbg_uit", [128, 512], F32, kind="ExternalOutput")
        dbg_tanh = nc.dram_tensor("dbg_tanh", [128, 512], F32, kind="ExternalOutput")
        dbg_sm = nc.dram_tensor("dbg_sm", [2, 128, NCH], F32, kind="ExternalOutput")
        dbg_ebf = nc.dram_tensor("dbg_ebf", [2, 128, NCH], F32, kind="ExternalOutput")
        dbg_uit5 = nc.dram_tensor("dbg_uit5", [128, 512], F32, kind="ExternalOutput")
        dbg_o2 = nc.dram_tensor("dbg_o2", [2, 2, 2 * D], F32, kind="ExternalOutput")
        dbg_den = nc.dram_tensor("dbg_den", [2, 1, 1], F32, kind="ExternalOutput")

    with tile.TileContext(nc) as tc:
        with (
            tc.tile_pool(name="const", bufs=1) as cpool,
            tc.tile_pool(name="xb", bufs=3) as xbpool,
            tc.tile_pool(name="xt", bufs=5) as xtpool,
            tc.tile_pool(name="th", bufs=3) as thpool,
            tc.tile_pool(name="small", bufs=2) as spool,
            tc.tile_pool(name="xtps", bufs=2, space="PSUM") as xtpspool,
            tc.tile_pool(name="uitps", bufs=2, space="PSUM") as uitpool,
            tc.tile_pool(name="eps", bufs=1, space="PSUM") as epool,
            tc.tile_pool(name="ops", bufs=1, space="PSUM") as opool,
            tc.tile_pool(name="o2ps", bufs=1, space="PSUM") as o2pool,
            tc.tile_pool(name="denps", bufs=1, space="PSUM") as denpool,
        ):
            # ---- constants (one-time) ----
            w2_f = cpool.tile([128, 2 * A], F32, name="w2_f")
            nc.sync.dma_start(out=w2_f[:], in_=w2_dram[:])
            w2_bf = cpool.tile([128, 2 * A], BF16, name="w2_bf")
            nc.vector.tensor_copy(w2_bf[:], w2_f[:])

            u4_f = cpool.tile([128, NQ], F32, name="u4_f")
            nc.sync.dma_start(out=u4_f[:], in_=u4_dram[:])
            u4_bf = cpool.tile([128, NQ], BF16, name="u4_bf")
            nc.vector.tensor_copy(u4_bf[:], u4_f[:])

            sel2 = cpool.tile([128, 2], F32, name="sel2")
            nc.sync.dma_start(out=sel2[:], in_=sel2_dram[:])

            b4 = cpool.tile([128, 1], F32, name="b4")
            nc.sync.dma_start(out=b4[:], in_=b4_dram[:])

            ident = cpool.tile([128, 128], F32, name="ident")
            nc.sync.dma_start(out=ident[:], in_=ident_dram[:])
            ident_bf = cpool.tile([128, 128], BF16, name="ident_bf")
            nc.vector.tensor_copy(ident_bf[:], ident[:])

            ones_f = cpool.tile([128, 1], F32, name="ones_f")
            nc.vector.memset(ones_f[:], 1.0)

            # deferred-emission state for software pipelining
            pend_score = [None]   # (tanh_sb, bb, g)
            e_tiles = {}          # bb -> e_ps tile, allocated at first write

            def emit_score(work):
                tanh_sb, bb, g = work
                # allocate e_ps(bb) lazily HERE so every reference to the
                # previous batch's e_ps tile is already emitted when the pool
                # rotates (bufs=1) -- else the tail's read races next scores.
                if bb not in e_tiles:
                    e_tiles[bb] = epool.tile([128, NCH], F32, name="e_ps", tag="e")
                e_v = e_tiles[bb].rearrange("p (g q c) -> p g q c", g=NG, q=NQ)
                for c in range(4):
                    nc.tensor.matmul(
                        e_v[:, g, :, c],
                        lhsT=tanh_sb[:, ds(128 * c, 128)],
                        rhs=u4_bf[:],
                        start=True,
                        stop=True,
                    )

            def emit_batch_dma(bb):
                """One cast-DMA per batch: [128, 32 chunks, 256] bf16,
                chunk col = 16g + r, t = 2048g + 16p + r."""
                x_bf = xbpool.tile([128, NG, RPG, D], BF16, name="x_bf", tag="xb")
                x_view = x_dram[bb].rearrange(
                    "(g p r) d -> p g r d", g=NG, p=128
                )
                if bb == 0:
                    for g in range(NG):
                        for k in range(4):
                            nc.gpsimd.dma_start(
                                out=x_bf[:, g, ds(4 * k, 4), :],
                                in_=x_view[:, g, ds(4 * k, 4), :],
                            )
                else:
                    nc.gpsimd.dma_start(out=x_bf[:], in_=x_view)
                return x_bf

            def emit_sg(bb, g, x_bf):
                """One supergroup: transposes, x@W, tanh."""
                uit_ps = uitpool.tile([128, 4 * 128], F32, name="uit_ps", tag="uit")
                # pair-trick transposes: one f32 [128,128] PE transpose moves a
                # whole [128t, 256d] bf16 chunk (adjacent bf16 pairs ride as one
                # f32; fp32r mantissa loss is ~2^-16 of the pair max -- far
                # below bf16 noise).  xt[P, 2s+j] = x[t_s, 2P+j].
                x_pair = x_bf.rearrange("p g r d -> p (g r d)").bitcast(F32)
                xt_sbs = []
                for q in range(NQ):
                    xt_ps = xtpspool.tile([128, 4, 128], F32, name="xt_ps", tag="xtps")
                    for cc in range(4):
                        r = RPG * g + 4 * q + cc
                        nc.tensor.transpose(
                            xt_ps[:, cc, :], x_pair[:, ds(128 * r, 128)], ident[:]
                        )
                    xt_sb = xtpool.tile([128, 4, 128], F32, name="xt_sb", tag="xt")
                    if q % 2 == 0:
                        nc.vector.tensor_copy(xt_sb[:], xt_ps[:])
                    else:
                        nc.scalar.copy(xt_sb[:], xt_ps[:])
                    xt_sbs.append(xt_sb[:].bitcast(BF16))  # [128, 4, 256]

                # deferred from previous SG: its score matmuls (tanh is ready)
                if pend_score[0] is not None:
                    emit_score(pend_score[0])

                # x@W: quad q accumulates uit^T into PSUM partitions 32q via
                # tile_position col-groups; q is the inner loop so the four
                # col-groups stream concurrently.  The start clear resets
                # has_written for the WRITTEN PARTITIONS across the full bank
                # width, so each quad's first matmul must set start=True.
                for j in range(2):
                    for cc in range(4):
                        for q in range(NQ):
                            nc.tensor.matmul(
                                uit_ps[ds(32 * q, 32), ds(128 * cc, 128)],
                                lhsT=w2_bf[:, ds(A * j, A)],
                                rhs=xt_sbs[q][:, cc, ds(j, 128, step=2)],
                                start=(j == 0 and cc == 0),
                                stop=(j == 1 and cc == 3 and q == NQ - 1),
                                tile_position=(0, 32 * q),
                            )

                tanh_sb = thpool.tile([128, 4 * 128], BF16, name="tanh_sb", tag="th")
                nc.scalar.activation(
                    tanh_sb[:],
                    uit_ps[:],
                    mybir.ActivationFunctionType.Tanh,
                    bias=b4[:],
                )
                if DEBUG and bb == 0 and g == 0:
                    t1 = spool.tile([128, 512], F32, name="dbg1", tag="dbg1")
                    nc.vector.tensor_copy(
                        t1[:], xt_sbs[0].bitcast(F32).rearrange("p c s -> p (c s)")
                    )
                    nc.sync.dma_start(out=dbg_xt[:], in_=t1[:])
                    t2 = spool.tile([128, 512], F32, name="dbg2", tag="dbg2")
                    nc.vector.tensor_copy(t2[:], uit_ps[:])
                    nc.sync.dma_start(out=dbg_uit[:], in_=t2[:])
                    t3 = spool.tile([128, 512], F32, name="dbg3", tag="dbg3")
                    nc.vector.tensor_copy(t3[:], tanh_sb[:])
                    nc.sync.dma_start(out=dbg_tanh[:], in_=t3[:])
                if DEBUG and bb == 5 and g == 0:
                    t2b = spool.tile([128, 512], F32, name="dbg2b", tag="dbg2b")
                    nc.vector.tensor_copy(t2b[:], uit_ps[:])
                    nc.sync.dma_start(out=dbg_uit5[:], in_=t2b[:])
                pend_score[0] = (tanh_sb, bb, g)
                return x_bf

            def emit_tail(bb, x_bf):
                e_ps = e_tiles.pop(bb)
                maskb = spool.tile([128, NCH], F32, name="maskb", tag="maskb")
                nc.sync.dma_start(out=maskb[:], in_=maskb_dram[bb])
                s_m = spool.tile([128, NCH], F32, name="s_m", tag="s_m")
                nc.vector.tensor_add(s_m[:], e_ps[:], maskb[:])

                e_bf = spool.tile([128, NCH], BF16, name="e_bf", tag="e_bf")
                er = spool.tile([128, 1], F32, name="er", tag="er")
                nc.scalar.activation(
                    e_bf[:],
                    s_m[:],
                    mybir.ActivationFunctionType.Exp,
                    accum_out=er[:],
                )
                if DEBUG and bb in (0, 5):
                    ii = 0 if bb == 0 else 1
                    nc.sync.dma_start(out=dbg_sm[ii], in_=s_m[:])
                    t5 = spool.tile([128, NCH], F32, name="dbg5", tag="dbg5")
                    nc.vector.tensor_copy(t5[:], e_bf[:])
                    nc.sync.dma_start(out=dbg_ebf[ii], in_=t5[:])

                den_ps = denpool.tile([1, 1], F32, name="den_ps", tag="den")
                nc.tensor.matmul(
                    den_ps[:], lhsT=er[:], rhs=ones_f[:], start=True, stop=True
                )

                # phase 3: weighted sum, two chunks per matmul, four
                # col-groups concurrent; region cg holds partial sums at
                # partitions {32cg, 32cg+1}.  sel2 matmul folds the four
                # regions down to the [2, 512] diagonal-block layout.
                o_ps = opool.tile([128, 2 * D], F32, name="o_ps", tag="o")
                for Q in range(NCH // 2):
                    cg = Q % 4
                    g3, r3 = divmod(2 * Q, RPG)
                    nc.tensor.matmul(
                        o_ps[ds(32 * cg, 2), :],
                        lhsT=e_bf[:, ds(2 * Q, 2)],
                        rhs=x_bf[:, g3, ds(r3, 2), :],
                        start=(Q < 4),
                        stop=(Q >= NCH // 2 - 4),
                        tile_position=(0, 32 * cg),
                    )
                o2f = spool.tile([128, 2 * D], F32, name="o2f", tag="o2f")
                nc.vector.tensor_copy(o2f[:], o_ps[:])
                o2_ps = o2pool.tile([2, 2 * D], F32, name="o2_ps", tag="o2")
                nc.tensor.matmul(
                    o2_ps[:], lhsT=sel2[:], rhs=o2f[:], start=True, stop=True
                )

                den_sb = spool.tile([1, 1], F32, name="den_sb", tag="den_sb")
                nc.vector.tensor_scalar_add(den_sb[:], den_ps[:], EPS)
                inv = spool.tile([1, 1], F32, name="inv", tag="inv")
                nc.vector.reciprocal(inv[:], den_sb[:])
                o2_sb = spool.tile([2, 2 * D], F32, name="o2_sb", tag="o2_sb")
                nc.vector.tensor_copy(o2_sb[:], o2_ps[:])
                o_hi = spool.tile([1, D], F32, name="o_hi", tag="o_hi")
                nc.sync.dma_start(out=o_hi[:], in_=o2_sb[1:2, ds(D, D)])
                o_sum = spool.tile([1, D], F32, name="o_sum", tag="o_sum")
                nc.vector.tensor_add(o_sum[:], o2_sb[0:1, 0:D], o_hi[:])
                o_sb = spool.tile([1, D], F32, name="o_sb", tag="o_sb")
                nc.vector.tensor_scalar_mul(o_sb[:], o_sum[:], inv[:])
                nc.sync.dma_start(out=out_dram[bb][None, :], in_=o_sb[:])
                if DEBUG and bb in (0, 5):
                    ii = 0 if bb == 0 else 1
                    nc.sync.dma_start(out=dbg_o2[ii], in_=o2_sb[:])
                    nc.sync.dma_start(out=dbg_den[ii], in_=den_sb[:])

            # pipeline: SG(bb,0) [emits scores of bb-1,g1] -> tail(bb-1)
            # -> SG(bb,1) [allocates e_ps(bb), emits scores of bb,g0]
            prev_xb = None
            for bb in range(BPC):
                x_bf = emit_batch_dma(bb)
                emit_sg(bb, 0, x_bf)
                if prev_xb is not None:
                    emit_tail(bb - 1, prev_xb)
                emit_sg(bb, 1, x_bf)
                prev_xb = x_bf
            emit_score(pend_score[0])
            emit_tail(BPC - 1, prev_xb)

    nc.finalize()
    return nc


def kernel(x, mask, W, b, u):
    global last_exec_time_ns, last_result
    x = np.ascontiguousarray(np.asarray(x), dtype=np.float32)
    mask_f = np.asarray(mask).astype(np.float32)
    W = np.asarray(W, dtype=np.float32)
    b = np.asarray(b, dtype=np.float32)
    u = np.asarray(u, dtype=np.float32)

    # host-side layout prep (all tiny; x is only view-sliced)
    # d-parity packing to match pair-trick transposes:
    # w2[p, A*j + a] = W[2p + j, a]
    w2 = np.ascontiguousarray(W.reshape(128, 2 * A))
    # block-diagonal u: u4[32q + a, q] = u[a]
    u4 = np.zeros((128, NQ), dtype=np.float32)
    for q in range(NQ):
        u4[32 * q : 32 * q + 32, q] = u[:, 0]
    # bias replicated per quad row-block
    b4 = np.ascontiguousarray(np.tile(b, NQ)[:, None])
    # mask -> additive pre-exp bias, laid out [b][p][(g r)] with t = 2048g+16p+r
    maskb = np.ascontiguousarray(
        ((mask_f - 1.0) * MASK_BIAS)
        .reshape(B, NG, 128, RPG)
        .transpose(0, 2, 1, 3)
        .reshape(B, 128, NCH)
    )
    ident = np.eye(128, dtype=np.float32)
    sel2 = np.zeros((128, 2), dtype=np.float32)
    for jj in range(4):
        sel2[32 * jj, 0] = 1.0
        sel2[32 * jj + 1, 1] = 1.0

    nc = _build()

    in_maps = []
    for c in range(N_CORES):
        in_maps.append(
            {
                "x": x[c * BPC : (c + 1) * BPC],
                "w2": w2,
                "u4": u4,
                "b4": b4,
                "maskb": maskb[c * BPC : (c + 1) * BPC],
                "ident": ident,
                "sel2": sel2,
            }
        )

    trace = bool(int(os.environ.get("BASS_KERNEL_TRACE", "0")))
    res = run_bass_kernel_spmd(
        nc, in_maps, core_ids=list(range(N_CORES)), trace=trace
    )
    last_exec_time_ns = res.exec_time_ns
    last_result = res

    out = np.empty((B, D), dtype=np.float32)
    for c in range(N_CORES):
        out[c * BPC : (c + 1) * BPC] = res.results[c]["out"]
    return out


# revision 27
# speedup vs baseline: 1.1215x; 1.0077x over previous
"""Attention-pooling kernel (AttLayer) for Trainium2, data-parallel over batch
across 8 NeuronCores.

  uit = tanh(x @ W + b)            [B, T, A]
  ait = exp(uit @ u) * mask        [B, T]
  out = einsum('btd,bt->bd', x, ait / (sum_t ait + eps))

Shapes hardcoded: x [64, 4096, 256] f32, W [256, 32], b [32], u [32, 1],
mask [64, 4096] bool. Each core handles 8 batches.

v2 design (PE-lean, DMA-bound target):
- x loads are SWDGE cast-DMAs (f32 DRAM -> bf16 SBUF), one [128,16,256]
  slab per half-batch "supergroup" (SG, 2048 t). t = 2048 g + 16 p + r.
- Per chunk (g, r): two bf16 [128,128] PE transposes -> xt (d on partitions).
- x@W uses W as the *stationary* operand: per quad q (4 chunks), matmuls
  write uit^T [32 a, 512 (c s)] at PSUM partitions 32q via
  tile_position=(0, 32q), accumulating two d-halves. One SG fills a full
  [128, 512] PSUM tile (4 quads stacked), so tanh(+bias b4) runs on all
  128 partitions in ONE activation op.
- Scores: per c, matmul(lhsT=tanh_sb[:, 128c:+128], rhs=u4) contracts over
  a within each 32-row quad block (u4 is block-diagonal u), yielding
  [128 slot, 4 quad] -- t lands on partitions, exactly phase-3's layout.
- mask bias add (DVE), exp with fused row-sum (ACT), denominator via a
  [128,1] x [128,1] matmul, phase 3 and output identical to v1.
"""

import os
import sys

sys.path.insert(0, "/opt/trn_rl_repo")

import numpy as np

import concourse.bass as bass
import concourse.mybir as mybir
import concourse.tile as tile
from concourse import bacc
from concourse.bass import ds, ts
from concourse import bass_utils
from concourse.bass_utils import run_bass_kernel_spmd

F32 = mybir.dt.float32
BF16 = mybir.dt.bfloat16

N_CORES = 8
B, T, D, A = 64, 4096, 256, 32
BPC = B // N_CORES          # batches per core
NCH = T // 128              # 128-row chunks per batch (32)
NG = 2                      # supergroups (DMA slabs) per batch
RPG = NCH // NG             # chunks per supergroup (16)
NQ = 4                      # quads per supergroup
EPS = 1e-7
MASK_BIAS = 30.0            # additive pre-exp mask: s + (mask-1)*30

last_exec_time_ns = None
last_result = None


DEBUG = bool(int(os.environ.get("BASS_V2_DEBUG", "0")))


def _build():
    nc = bacc.Bacc(None, target_bir_lowering=False, debug=True)

    x_dram = nc.dram_tensor("x", [BPC, T, D], F32, kind="ExternalInput")
    w2_dram = nc.dram_tensor("w2", [128, 2 * A], F32, kind="ExternalInput")
    u4_dram = nc.dram_tensor("u4", [128, NQ], F32, kind="ExternalInput")
    sel2_dram = nc.dram_tensor("sel2", [128, 2], F32, kind="ExternalInput")
    b4_dram = nc.dram_tensor("b4", [128, 1], F32, kind="ExternalInput")
    maskb_dram = nc.dram_tensor("maskb", [BPC, 128, NCH], F32, kind="ExternalInput")
    ident_dram = nc.dram_tensor("ident", [128, 128], F32, kind="ExternalInput")
    out_dram = nc.dram_tensor("out", [BPC, D], F32, kind="ExternalOutput")
    if DEBUG:
        dbg_xt = nc.dram_tensor("dbg_xt", [128, 512], F32, kind="ExternalOutput")
        dbg_uit = nc.dram_tensor("dbg_uit", [128, 512], F32, kind="ExternalOutput")
        dbg_tanh = nc.dram_tensor("dbg_tanh", [128, 512], F32, kind="ExternalOutput")
        dbg_sm = nc.dram_tensor("dbg_sm", [2, 128, NCH], F32, kind="ExternalOutput")
        dbg_ebf = nc.dram_tensor("dbg_ebf", [2, 128, NCH], F32, kind="ExternalOutput")
        dbg_uit5 = nc.dram_tensor("dbg_uit5", [128, 512], F32, kind="ExternalOutput")
        dbg_o2 = nc.dram_tensor("dbg_o2", [2, 2, 2 * D], F32, kind="ExternalOutput")
        dbg_den = nc.dram_tensor("dbg_den", [2, 1, 1], F32, kind="ExternalOutput")

    with tile.TileContext(nc) as tc:
        with (
            tc.tile_pool(name="const", bufs=1) as cpool,
            tc.tile_pool(name="xb", bufs=3) as xbpool,
            tc.tile_pool(name="xt", bufs=5) as xtpool,
            tc.tile_pool(name="th", bufs=3) as thpool,
            tc.tile_pool(name="small", bufs=2) as spool,
            tc.tile_pool(name="xtps", bufs=2, space="PSUM") as xtpspool,
            tc.tile_pool(name="uitps", bufs=2, space="PSUM") as uitpool,
            tc.tile_pool(name="eps", bufs=1, space="PSUM") as epool,
            tc.tile_pool(name="ops", bufs=1, space="PSUM") as opool,
            tc.tile_pool(name="o2ps", bufs=1, space="PSUM") as o2pool,
            tc.tile_pool(name="denps", bufs=1, space="PSUM") as denpool,
        ):
            # ---- constants (one-time) ----
            w2_f = cpool.tile([128, 2 * A], F32, name="w2_f")
            nc.sync.dma_start(out=w2_f[:], in_=w2_dram[:])
            w2_bf = cpool.tile([128, 2 * A], BF16, name="w2_bf")
            nc.vector.tensor_copy(w2_bf[:], w2_f[:])

            u4_f = cpool.tile([128, NQ], F32, name="u4_f")
            nc.sync.dma_start(out=u4_f[:], in_=u4_dram[:])
            u4_bf = cpool.tile([128, NQ], BF16, name="u4_bf")
            nc.vector.tensor_copy(u4_bf[:], u4_f[:])

            sel2 = cpool.tile([128, 2], F32, name="sel2")
            nc.sync.dma_start(out=sel2[:], in_=sel2_dram[:])

            b4 = cpool.tile([128, 1], F32, name="b4")
            nc.sync.dma_start(out=b4[:], in_=b4_dram[:])

            ident = cpool.tile([128, 128], F32, name="ident")
            nc.sync.dma_start(out=ident[:], in_=ident_dram[:])
            ident_bf = cpool.tile([128, 128], BF16, name="ident_bf")
            nc.vector.tensor_copy(ident_bf[:], ident[:])

            ones_f = cpool.tile([128, 1], F32, name="ones_f")
            nc.vector.memset(ones_f[:], 1.0)

            # deferred-emission state for software pipelining
            pend_score = [None]   # (tanh_sb, bb, g)
            e_tiles = {}          # bb -> e_ps tile, allocated at first write
            den_tiles = {}        # bb -> den psum tile
            o_tiles = {}          # bb -> phase-3 psum tile
            mb_tiles = {}         # bb -> maskb tile
    
            def emit_score(work):
                tanh_sb, bb, g = work
                # allocate e_ps(bb) lazily HERE so every reference to the
                # previous batch's e_ps tile is already emitted when the pool
                # rotates (bufs=1) -- else the tail's read races next scores.
                if bb not in e_tiles:
                    e_tiles[bb] = epool.tile([128, NCH], F32, name="e_ps", tag="e")
                e_v = e_tiles[bb].rearrange("p (g q c) -> p g q c", g=NG, q=NQ)
                for c in range(4):
                    nc.tensor.matmul(
                        e_v[:, g, :, c],
                        lhsT=tanh_sb[:, ds(128 * c, 128)],
                        rhs=u4_bf[:],
                        start=True,
                        stop=True,
                    )
                emit_sg_tail(bb, g)
                if g == 1:
                    emit_batch_final(bb)

            def emit_sg_tail(bb, g):
                """Mask+exp+den+phase-3 for the 16 chunks of SG (bb, g)."""
                e_ps = e_tiles[bb]
                x_bf = xb_tiles[bb]
                s_m = spool.tile([128, RPG], F32, name="s_m", tag="s_m")
                nc.vector.tensor_add(
                    s_m[:], e_ps[:, ds(RPG * g, RPG)],
                    mb_tiles[bb][:, ds(RPG * g, RPG)],
                )
                e_bf = spool.tile([128, RPG], BF16, name="e_bf", tag="e_bf")
                er = spool.tile([128, 1], F32, name="er", tag="er")
                nc.scalar.activation(
                    e_bf[:], s_m[:], mybir.ActivationFunctionType.Exp,
                    accum_out=er[:],
                )
                if g == 0:
                    den_tiles[bb] = denpool.tile([1, 1], F32, name="den_ps", tag="den")
                    o_tiles[bb] = opool.tile([128, 2 * D], F32, name="o_ps", tag="o")
                nc.tensor.matmul(
                    den_tiles[bb][:], lhsT=er[:], rhs=ones_f[:],
                    start=(g == 0), stop=(g == 1),
                )
                o_ps = o_tiles[bb]
                for k in range(RPG // 2):
                    cg = k % 4
                    nc.tensor.matmul(
                        o_ps[ds(32 * cg, 2), :],
                        lhsT=e_bf[:, ds(2 * k, 2)],
                        rhs=x_bf[:, g, ds(2 * k, 2), :],
                        start=(g == 0 and k < 4),
                        stop=(g == 1 and k >= 4),
                        tile_position=(0, 32 * cg),
                    )

            def emit_batch_final(bb):
                e_tiles.pop(bb)
                mb_tiles.pop(bb)
                xb_tiles.pop(bb)
                o_ps = o_tiles.pop(bb)
                den_ps = den_tiles.pop(bb)
                o2f = spool.tile([128, 2 * D], F32, name="o2f", tag="o2f")
                nc.vector.tensor_copy(o2f[:], o_ps[:])
                o2_ps = o2pool.tile([2, 2 * D], F32, name="o2_ps", tag="o2")
                nc.tensor.matmul(
                    o2_ps[:], lhsT=sel2[:], rhs=o2f[:], start=True, stop=True
                )
                den_sb = spool.tile([1, 1], F32, name="den_sb", tag="den_sb")
                nc.vector.tensor_scalar_add(den_sb[:], den_ps[:], EPS)
                inv = spool.tile([1, 1], F32, name="inv", tag="inv")
                nc.vector.reciprocal(inv[:], den_sb[:])
                o2_sb = spool.tile([2, 2 * D], F32, name="o2_sb", tag="o2_sb")
                nc.vector.tensor_copy(o2_sb[:], o2_ps[:])
                o_hi = spool.tile([1, D], F32, name="o_hi", tag="o_hi")
                nc.sync.dma_start(out=o_hi[:], in_=o2_sb[1:2, ds(D, D)])
                o_sum = spool.tile([1, D], F32, name="o_sum", tag="o_sum")
                nc.vector.tensor_add(o_sum[:], o2_sb[0:1, 0:D], o_hi[:])
                o_sb = spool.tile([1, D], F32, name="o_sb", tag="o_sb")
                nc.vector.tensor_scalar_mul(o_sb[:], o_sum[:], inv[:])
                nc.sync.dma_start(out=out_dram[bb][None, :], in_=o_sb[:])

            def emit_batch_dma(bb):
                """One cast-DMA per batch: [128, 32 chunks, 256] bf16,
                chunk col = 16g + r, t = 2048g + 16p + r."""
                x_bf = xbpool.tile([128, NG, RPG, D], BF16, name="x_bf", tag="xb")
                x_view = x_dram[bb].rearrange(
                    "(g p r) d -> p g r d", g=NG, p=128
                )
                if bb == 0:
                    for g in range(NG):
                        for k in range(4):
                            nc.gpsimd.dma_start(
                                out=x_bf[:, g, ds(4 * k, 4), :],
                                in_=x_view[:, g, ds(4 * k, 4), :],
                            )
                else:
                    nc.gpsimd.dma_start(out=x_bf[:], in_=x_view)
                maskb = spool.tile([128, NCH], F32, name="maskb", tag="maskb")
                nc.sync.dma_start(out=maskb[:], in_=maskb_dram[bb])
                mb_tiles[bb] = maskb
                xb_tiles[bb] = x_bf
                return x_bf

            def emit_sg(bb, g, x_bf):
                """One supergroup: transposes, x@W, tanh."""
                uit_ps = uitpool.tile([128, 4 * 128], F32, name="uit_ps", tag="uit")
                # pair-trick transposes: one f32 [128,128] PE transpose moves a
                # whole [128t, 256d] bf16 chunk (adjacent bf16 pairs ride as one
                # f32; fp32r mantissa loss is ~2^-16 of the pair max -- far
                # below bf16 noise).  xt[P, 2s+j] = x[t_s, 2P+j].
                x_pair = x_bf.rearrange("p g r d -> p (g r d)").bitcast(F32)
                xt_sbs = []
                for q in range(NQ):
                    xt_ps = xtpspool.tile([128, 4, 128], F32, name="xt_ps", tag="xtps")
                    for cc in range(4):
                        r = RPG * g + 4 * q + cc
                        nc.tensor.transpose(
                            xt_ps[:, cc, :], x_pair[:, ds(128 * r, 128)], ident[:]
                        )
                    xt_sb = xtpool.tile([128, 4, 128], F32, name="xt_sb", tag="xt")
                    if q % 2 == 0:
                        nc.vector.tensor_copy(xt_sb[:], xt_ps[:])
                    else:
                        nc.scalar.copy(xt_sb[:], xt_ps[:])
                    xt_sbs.append(xt_sb[:].bitcast(BF16))  # [128, 4, 256]

                # deferred from previous SG: its score matmuls (tanh is ready)
                if pend_score[0] is not None:
                    emit_score(pend_score[0])

                # x@W: quad q accumulates uit^T into PSUM partitions 32q via
                # tile_position col-groups; q is the inner loop so the four
                # col-groups stream concurrently.  The start clear resets
                # has_written for the WRITTEN PARTITIONS across the full bank
                # width, so each quad's first matmul must set start=True.
                for j in range(2):
                    for cc in range(4):
                        for q in range(NQ):
                            nc.tensor.matmul(
                                uit_ps[ds(32 * q, 32), ds(128 * cc, 128)],
                                lhsT=w2_bf[:, ds(A * j, A)],
                                rhs=xt_sbs[q][:, cc, ds(j, 128, step=2)],
                                start=(j == 0 and cc == 0),
                                stop=(j == 1 and cc == 3 and q == NQ - 1),
                                tile_position=(0, 32 * q),
                            )

                tanh_sb = thpool.tile([128, 4 * 128], BF16, name="tanh_sb", tag="th")
                nc.scalar.activation(
                    tanh_sb[:],
                    uit_ps[:],
                    mybir.ActivationFunctionType.Tanh,
                    bias=b4[:],
                )
                if DEBUG and bb == 0 and g == 0:
                    t1 = spool.tile([128, 512], F32, name="dbg1", tag="dbg1")
                    nc.vector.tensor_copy(
                        t1[:], xt_sbs[0].bitcast(F32).rearrange("p c s -> p (c s)")
                    )
                    nc.sync.dma_start(out=dbg_xt[:], in_=t1[:])
                    t2 = spool.tile([128, 512], F32, name="dbg2", tag="dbg2")
                    nc.vector.tensor_copy(t2[:], uit_ps[:])
                    nc.sync.dma_start(out=dbg_uit[:], in_=t2[:])
                    t3 = spool.tile([128, 512], F32, name="dbg3", tag="dbg3")
                    nc.vector.tensor_copy(t3[:], tanh_sb[:])
                    nc.sync.dma_start(out=dbg_tanh[:], in_=t3[:])
                if DEBUG and bb == 5 and g == 0:
                    t2b = spool.tile([128, 512], F32, name="dbg2b", tag="dbg2b")
                    nc.vector.tensor_copy(t2b[:], uit_ps[:])
                    nc.sync.dma_start(out=dbg_uit5[:], in_=t2b[:])
                pend_score[0] = (tanh_sb, bb, g)
                return x_bf

            # pipeline: the pend_score hook (fired inside the next SG's
            # emission) emits scores, then that SG's mask/exp/den/phase-3,
            # and on g==1 the batch finalization.
            xb_tiles = {}
            for bb in range(BPC):
                x_bf = emit_batch_dma(bb)
                emit_sg(bb, 0, x_bf)
                emit_sg(bb, 1, x_bf)
            emit_score(pend_score[0])

    nc.finalize()
    return nc


def kernel(x, mask, W, b, u):
    global last_exec_time_ns, last_result
    x = np.ascontiguousarray(np.asarray(x), dtype=np.float32)
    mask_f = np.asarray(mask).astype(np.float32)
    W = np.asarray(W, dtype=np.float32)
    b = np.asarray(b, dtype=np.float32)
    u = np.asarray(u, dtype=np.float32)

    # host-side layout prep (all tiny; x is only view-sliced)
    # d-parity packing to match pair-trick transposes:
    # w2[p, A*j + a] = W[2p + j, a]
    w2 = np.ascontiguousarray(W.reshape(128, 2 * A))
    # block-diagonal u: u4[32q + a, q] = u[a]
    u4 = np.zeros((128, NQ), dtype=np.float32)
    for q in range(NQ):
        u4[32 * q : 32 * q + 32, q] = u[:, 0]
    # bias replicated per quad row-block
    b4 = np.ascontiguousarray(np.tile(b, NQ)[:, None])
    # mask -> additive pre-exp bias, laid out [b][p][(g r)] with t = 2048g+16p+r
    maskb = np.ascontiguousarray(
        ((mask_f - 1.0) * MASK_BIAS)
        .reshape(B, NG, 128, RPG)
        .transpose(0, 2, 1, 3)
        .reshape(B, 128, NCH)
    )
    ident = np.eye(128, dtype=np.float32)
    sel2 = np.zeros((128, 2), dtype=np.float32)
    for jj in range(4):
        sel2[32 * jj, 0] = 1.0
        sel2[32 * jj + 1, 1] = 1.0

    nc = _build()

    in_maps = []
    for c in range(N_CORES):
        in_maps.append(
            {
                "x": x[c * BPC : (c + 1) * BPC],
                "w2": w2,
                "u4": u4,
                "b4": b4,
                "maskb": maskb[c * BPC : (c + 1) * BPC],
                "ident": ident,
                "sel2": sel2,
            }
        )

    trace = bool(int(os.environ.get("BASS_KERNEL_TRACE", "0")))
    res = run_bass_kernel_spmd(
        nc, in_maps, core_ids=list(range(N_CORES)), trace=trace
    )
    last_exec_time_ns = res.exec_time_ns
    last_result = res

    out = np.empty((B, D), dtype=np.float32)
    for c in range(N_CORES):
        out[c * BPC : (c + 1) * BPC] = res.results[c]["out"]
    return out
